# revision 2
# baseline (speedup 1.0000x reference)
"""Trainium2 Bass kernel for the AP-model RHS:
    out = concat(S @ u + 8*u*(1-u)*(u-par) - u*v,  -0.01*(8*u*(u-par-1) + v))
with D=8192, S row-sharded across 8 NeuronCores (1024 rows each).

v2 strategy — fp8 streaming (the kernel is HBM-bound, so bytes are the
whole game):
  - S is pre-quantized on the host to float8 e3m4 (4 mantissa bits) with a
    global scale of 128 (S values ~N(0, 1/8192); x128 puts them in e3m4's
    normal range).  End-to-end rel err ~1.2e-2 vs the 2e-2 gate (measured
    against the exact reference), with 4x less HBM traffic than f32:
    8 MB/core instead of 32 MB.
  - the host hands each core a PACKED TRANSPOSE of its row-shard:
    st[p, off_t + jl*1024 + m] = Sq[c*1024+m, (c0_t+jl)*128+p], so every
    DMA tile is a [128, nj*1024] column-slice with per-partition-contiguous
    lines; small tiles at both ends shorten pipeline fill/drain.
  - PE ingest is 1 moving column (128 values)/cycle regardless of dtype, so
    a plain fp8 matvec would be TensorE-bound (~27us > ~19us of DMA).  The
    k-chunks are therefore spread round-robin over 3 PE column groups
    (tile_position derived from the PSUM accumulator partition base 32*g),
    whose matmuls execute concurrently on disjoint 32-column strips of the
    128x128 array, each with its own XBUS stream.
  - u rides as the stationary operand in bf16, pre-scaled by 1/128 so the
    PSUM result needs no rescale.  (bf16 lhsT x fp8 rhs mixed matmul.)
  - reaction terms are a handful of [1, 1024] VectorE ops overlapped with
    the stream; the 3 group partials + reaction fold in a short tail whose
    first adds are hidden by staggering the groups' last chunks.
"""

import numpy as np
import ml_dtypes

import concourse.bacc as bacc
import concourse.mybir as mybir
import concourse.tile as tile
from concourse.bass_utils import run_bass_kernel_spmd

D = 8192
N_CORES = 8
ROWS = D // N_CORES          # 1024 rows of S per core
NKC = D // 128               # 64 k-chunks of 128
F32 = mybir.dt.float32
F8 = mybir.dt.float8e3      # e3m4
BF16 = mybir.dt.bfloat16
K_PARAM = 8.0
EPS_PARAM = 0.01

S_SCALE = 128.0              # S quantized as e3m4(S * 128); folded into u

NGRP = 3                     # PE column groups (avoid quadrant 3)
# chunk counts per DMA tile: small at the ends for pipeline fill/drain
TILE_CHUNKS = [4, 4, 8, 16, 16, 8, 4, 4]
assert sum(TILE_CHUNKS) == NKC

# mixed-precision stationary: bf16 u against fp8 S.  If False, u is fed as
# two e3m4 columns (hi+lo) and rescaled in the tail.
USE_MIXED_U = True
U_HI_SCALE = 8.0
U_LO_SCALE = 512.0

_CACHE = {}


def _emit_body(nc, pools, st_ext, u_ext, loc_ext, out_ext):
    mult = mybir.AluOpType.mult
    add = mybir.AluOpType.add
    sub = mybir.AluOpType.subtract
    big_pool, small_pool, psum_pool = pools

    acc = psum_pool.tile([128, ROWS], F32, tag="acc")

    u_cols = 1 if USE_MIXED_U else 2
    u_sb = small_pool.tile([128, u_cols * NKC], BF16 if USE_MIXED_U else F8,
                           tag="u")
    nc.scalar.dma_start(out=u_sb[:], in_=u_ext[:])
    loc_sb = small_pool.tile([1, 3 * ROWS], F32, tag="loc")
    nc.scalar.dma_start(out=loc_sb[:], in_=loc_ext[:])

    # group bookkeeping: chunk j -> group j % NGRP
    first_of_g = [min(j for j in range(NKC) if j % NGRP == g)
                  for g in range(NGRP)]
    last_of_g = [max(j for j in range(NKC) if j % NGRP == g)
                 for g in range(NGRP)]

    col_off = 0
    for nj in TILE_CHUNKS:
        s_tile = big_pool.tile([128, nj * ROWS], F8, tag=f"s{nj}")
        nc.sync.dma_start(out=s_tile[:],
                          in_=st_ext[:, col_off * ROWS:(col_off + nj) * ROWS])
        for jl in range(nj):
            j = col_off + jl
            g = j % NGRP
            base = 32 * g
            for h in range(2):
                nc.tensor.matmul(
                    acc[base:base + u_cols, h * 512:(h + 1) * 512],
                    lhsT=u_sb[:, u_cols * j:u_cols * (j + 1)],
                    rhs=s_tile[:, jl * ROWS + h * 512: jl * ROWS + (h + 1) * 512],
                    start=(j == first_of_g[g]),
                    stop=(j == last_of_g[g]),
                )
        col_off += nj

    # --- reaction terms on [1, 1024] tiles (DVE), overlapped w/ the stream
    u_t = loc_sb[0:1, 0:ROWS]
    v_t = loc_sb[0:1, ROWS:2 * ROWS]
    par_t = loc_sb[0:1, 2 * ROWS:3 * ROWS]
    out_sb = small_pool.tile([1, 2 * ROWS], F32, tag="osb")
    s1 = small_pool.tile([1, ROWS], F32, tag="s1")
    s2 = small_pool.tile([1, ROWS], F32, tag="s2")
    s3 = small_pool.tile([1, ROWS], F32, tag="s3")

    nc.vector.tensor_tensor(out=s1[:], in0=u_t, in1=par_t, op=sub)      # u-par
    nc.vector.tensor_scalar_sub(out=s2[:], in0=s1[:], scalar1=1.0)      # u-par-1
    nc.vector.tensor_tensor(out=s2[:], in0=u_t, in1=s2[:], op=mult)     # u(u-par-1)
    nc.vector.tensor_scalar_mul(out=s2[:], in0=s2[:],
                                scalar1=-K_PARAM * EPS_PARAM)
    nc.vector.tensor_scalar_mul(out=s3[:], in0=v_t, scalar1=EPS_PARAM)  # .01v
    nc.vector.tensor_tensor(out=out_sb[0:1, ROWS:2 * ROWS],
                            in0=s2[:], in1=s3[:], op=sub)               # pde2
    nc.vector.tensor_tensor(out=s2[:], in0=u_t, in1=u_t, op=mult)       # u^2
    nc.vector.tensor_tensor(out=s2[:], in0=u_t, in1=s2[:], op=sub)      # u(1-u)
    nc.vector.tensor_tensor(out=s2[:], in0=s2[:], in1=s1[:], op=mult)
    nc.vector.tensor_tensor(out=s3[:], in0=u_t, in1=v_t, op=mult)       # uv
    # s2 = 8*s2 - s3   (= the reaction part of pde1)
    nc.vector.scalar_tensor_tensor(out=s2[:], in0=s2[:], scalar=K_PARAM,
                                   in1=s3[:], op0=mult, op1=sub)

    # --- tail: fold the NGRP group partials (+ u hi/lo rows) into pde1.
    # Groups finish in order g = (NKC-NGRP..NKC-1) % NGRP; combine in that
    # order so earlier adds hide under the remaining stream.
    t = small_pool.tile([1, ROWS], F32, tag="t")
    order = [(NKC - NGRP + i) % NGRP for i in range(NGRP)]
    prev = s2[:]
    if USE_MIXED_U:
        for g in order:
            dst = out_sb[0:1, 0:ROWS] if g == order[-1] else t[:]
            nc.vector.tensor_tensor(out=dst, in0=acc[32 * g:32 * g + 1, :],
                                    in1=prev, op=add)
            prev = t[:]
    else:
        c_hi = 1.0 / (S_SCALE * U_HI_SCALE)
        c_lo = 1.0 / (S_SCALE * U_LO_SCALE)
        for g in order:
            nc.vector.scalar_tensor_tensor(
                out=t[:], in0=acc[32 * g + 1:32 * g + 2, :], scalar=c_lo,
                in1=prev, op0=mult, op1=add)
            dst = out_sb[0:1, 0:ROWS] if g == order[-1] else t[:]
            nc.vector.scalar_tensor_tensor(
                out=dst, in0=acc[32 * g:32 * g + 1, :], scalar=c_hi,
                in1=t[:], op0=mult, op1=add)
            prev = t[:]

    nc.sync.dma_start(out=out_ext[:], in_=out_sb[:])


def build_nc(reps=1):
    nc = bacc.Bacc("TRN2", target_bir_lowering=False, debug=False,
                   num_devices=N_CORES)

    st_ext = nc.dram_tensor("st", [128, NKC * ROWS], F8, kind="ExternalInput")
    u_cols = 1 if USE_MIXED_U else 2
    u_ext = nc.dram_tensor("uq", [128, u_cols * NKC],
                           BF16 if USE_MIXED_U else F8, kind="ExternalInput")
    loc_ext = nc.dram_tensor("loc", [1, 3 * ROWS], F32, kind="ExternalInput")
    out_ext = nc.dram_tensor("out", [1, 2 * ROWS], F32, kind="ExternalOutput")

    with tile.TileContext(nc, pool_alloc_mode="queue") as tc:
        with (
            tc.tile_pool(name="big_pool", bufs=4) as big_pool,
            tc.tile_pool(name="small", bufs=1) as small_pool,
            tc.tile_pool(name="psum", bufs=2, space="PSUM") as psum_pool,
        ):
            for _rep in range(reps):
                _emit_body(nc, (big_pool, small_pool, psum_pool),
                           st_ext, u_ext, loc_ext, out_ext)

    nc.compile()
    return nc


def _get_nc():
    if "nc" not in _CACHE:
        _CACHE["nc"] = build_nc()
    return _CACHE["nc"]


def make_in_maps(y, S, par):
    u = y[:D]
    v = y[D:2 * D]
    par_flat = par.reshape(-1)

    if USE_MIXED_U:
        uq = np.ascontiguousarray(
            (u / S_SCALE).reshape(NKC, 128).T).astype(ml_dtypes.bfloat16)
    else:
        u_hi = (u * U_HI_SCALE).astype(ml_dtypes.float8_e3m4)
        u_lo = ((u - u_hi.astype(np.float32) / U_HI_SCALE)
                * U_LO_SCALE).astype(ml_dtypes.float8_e3m4)
        uq = np.empty((128, 2 * NKC), ml_dtypes.float8_e3m4)
        uq[:, 0::2] = u_hi.reshape(NKC, 128).T
        uq[:, 1::2] = u_lo.reshape(NKC, 128).T
        uq = np.ascontiguousarray(uq)

    in_maps = []
    for c in range(N_CORES):
        sl = slice(c * ROWS, (c + 1) * ROWS)
        Sq = (S[sl] * S_SCALE).astype(ml_dtypes.float8_e3m4)
        # st[p, j*1024 + m] = Sq[m, j*128 + p]
        st = np.ascontiguousarray(
            Sq.T.reshape(NKC, 128, ROWS).transpose(1, 0, 2).reshape(
                128, NKC * ROWS))
        loc = np.concatenate([u[sl], v[sl], par_flat[sl]]).reshape(1, 3 * ROWS)
        in_maps.append({
            "st": st,
            "uq": uq,
            "loc": np.ascontiguousarray(loc.astype(np.float32)),
        })
    return in_maps


def assemble_output(results):
    full = np.empty(2 * D, np.float32)
    for c in range(N_CORES):
        o = results[c]["out"][0]         # [2048]
        full[c * ROWS:(c + 1) * ROWS] = o[0:ROWS]
        full[D + c * ROWS:D + (c + 1) * ROWS] = o[ROWS:2 * ROWS]
    return full


def kernel(t=None, y=None, S=None, par=None, **_unused):
    y = np.asarray(y, np.float32)
    S = np.asarray(S, np.float32)
    par = np.asarray(par, np.float32)
    nc = _get_nc()
    in_maps = make_in_maps(y, S, par)
    res = run_bass_kernel_spmd(nc, in_maps, core_ids=list(range(N_CORES)))
    return assemble_output(res.results)
